# revision 1
# baseline (speedup 1.0000x reference)
"""Groupwise asymmetric 4-bit quantize+dequantize (KV-cache RTN) on 8 TRN2 cores.

Reference semantics (per contiguous group of 128 along the last dim):
  scale  = max((max(g) - min(g)) / 15, 1e-8)
  offset = round(-min(g) / scale)
  q      = clip(round(x / scale) + offset, 0, 15)
  out    = (q - offset) * scale

Kernel formulation (provably equivalent up to reciprocal-vs-divide ulps):
  rscale = 1 / scale
  u      = round(x * rscale)            # ACT Copy with int32 output (RNE)
  hi     = round(min(g) * rscale) + 15  # lower clamp never fires (monotonicity)
  out    = min(u, hi) * scale           # fused vector tensor_scalar

Sharding: fully elementwise per group -> split the flat tensor into 8 equal
contiguous shards, one per NeuronCore, no communication.
"""

import sys

sys.path.insert(0, "/opt/trn_rl_repo")

import numpy as np

import concourse.bass as bass  # noqa: F401  (engine types referenced via nc)
import concourse.bacc as bacc
import concourse.mybir as mybir
import concourse.tile as tile
from concourse.bass_utils import run_bass_kernel_spmd

# Problem constants (hardcoded per harness contract)
FULL_SHAPE = (4, 32, 4096, 128)
N_CORES = 8
G = 128                      # group size (elements per quant group)
TOTAL = 4 * 32 * 4096 * 128  # 67,108,864 elements
PER_CORE = TOTAL // N_CORES  # 8,388,608 elements
GROUPS_PER_CORE = PER_CORE // G  # 65,536 groups

P = 128                      # SBUF partitions
F = 16                       # groups per partition per tile
TILE_GROUPS = P * F          # 2048 groups per tile
TILE_FREE = F * G            # 2048 elements per partition per tile
N_TILES = GROUPS_PER_CORE // TILE_GROUPS  # 32

M = 12582912.0               # 1.5 * 2**23 (round-to-int magic constant)

_COMPILED = None


def _build():
    nc = bacc.Bacc("TRN2", target_bir_lowering=False, debug=False)
    x_d = nc.dram_tensor(
        "x", [GROUPS_PER_CORE, G], mybir.dt.float32, kind="ExternalInput"
    ).ap()
    y_d = nc.dram_tensor(
        "y", [GROUPS_PER_CORE, G], mybir.dt.float32, kind="ExternalOutput"
    ).ap()

    with tile.TileContext(nc) as tc:
        with (
            tc.tile_pool(name="xp", bufs=3) as xp,
            tc.tile_pool(name="up", bufs=3) as up,
            tc.tile_pool(name="op", bufs=3) as op,
            tc.tile_pool(name="st", bufs=4) as st,
        ):
            for t in range(N_TILES):
                rows = x_d[t * TILE_GROUPS : (t + 1) * TILE_GROUPS, :]
                xt = xp.tile([P, TILE_FREE], mybir.dt.float32, tag="x")
                nc.sync.dma_start(out=xt[:], in_=rows.rearrange("(p f) g -> p (f g)", p=P))

                x3 = xt[:].rearrange("p (f g) -> p f g", g=G)
                mx = st.tile([P, F], mybir.dt.float32, tag="mx")
                mn = st.tile([P, F], mybir.dt.float32, tag="mn")
                nc.vector.tensor_reduce(
                    mx[:], x3, axis=mybir.AxisListType.X, op=mybir.AluOpType.max
                )
                nc.vector.tensor_reduce(
                    mn[:], x3, axis=mybir.AxisListType.X, op=mybir.AluOpType.min
                )

                sc = st.tile([P, F], mybir.dt.float32, tag="sc")
                nc.vector.tensor_tensor(sc[:], mx[:], mn[:], op=mybir.AluOpType.subtract)
                nc.vector.tensor_scalar(
                    sc[:], sc[:], 1.0 / 15.0, 1e-8,
                    op0=mybir.AluOpType.mult, op1=mybir.AluOpType.max,
                )
                rs = st.tile([P, F], mybir.dt.float32, tag="rs")
                nc.vector.reciprocal(rs[:], sc[:])
                hi = st.tile([P, F], mybir.dt.float32, tag="hi")
                nc.vector.tensor_tensor(hi[:], mn[:], rs[:], op=mybir.AluOpType.mult)
                nc.vector.tensor_scalar(
                    hi[:], hi[:], M, M - 15.0,
                    op0=mybir.AluOpType.add, op1=mybir.AluOpType.subtract,
                )

                ut = up.tile([P, TILE_FREE], mybir.dt.int32, tag="u")
                ot = op.tile([P, TILE_FREE], mybir.dt.float32, tag="o")
                for f in range(F):
                    s = slice(f * G, (f + 1) * G)
                    nc.scalar.activation(
                        ut[:, s], xt[:, s],
                        mybir.ActivationFunctionType.Copy,
                        bias=0.0, scale=rs[:, f : f + 1],
                    )
                    nc.vector.tensor_scalar(
                        ot[:, s], ut[:, s], hi[:, f : f + 1], sc[:, f : f + 1],
                        op0=mybir.AluOpType.min, op1=mybir.AluOpType.mult,
                    )

                orows = y_d[t * TILE_GROUPS : (t + 1) * TILE_GROUPS, :]
                nc.sync.dma_start(
                    out=orows.rearrange("(p f) g -> p (f g)", p=P), in_=ot[:]
                )

    nc.compile()
    return nc


def _get_compiled():
    global _COMPILED
    if _COMPILED is None:
        _COMPILED = _build()
    return _COMPILED


def kernel(x: np.ndarray) -> np.ndarray:
    assert x.shape == FULL_SHAPE and x.dtype == np.float32, (x.shape, x.dtype)
    nc = _get_compiled()
    flat = np.ascontiguousarray(x).reshape(N_CORES, GROUPS_PER_CORE, G)
    in_maps = [{"x": flat[i]} for i in range(N_CORES)]
    res = run_bass_kernel_spmd(nc, in_maps, core_ids=list(range(N_CORES)))
    out = np.empty((N_CORES, GROUPS_PER_CORE, G), dtype=np.float32)
    for i in range(N_CORES):
        out[i] = res.results[i]["y"]
    return out.reshape(FULL_SHAPE)
